# revision 9
# baseline (speedup 1.0000x reference)
"""8-core TRN2 Bass kernel for the 6-layer GCN edge classifier.

Architecture (per core, nodes dst-sharded 8 ways):
- hw' = (x_embed @ W_l) * dinv  computed locally, AllGather -> replicated table
- message aggregation: dma_gather (src-window int16-rebased chunks, 4 SWDGE
  queues) + dma_scatter_add into HBM accumulators; chunks are "pass"-structured
  so destination indices are unique within every scatter call (passes = j-th
  in-edge of each node), alternating 2 accumulators for pipelining.
- BN stats via on-chip reduction + tiny AllReduce; apply+relu+residual on DVE.
- final edge head: y = [xe@fcW_top | xe@fcW_bot] per node -> AllGather ->
  per-edge gathers; host adds the two halves + bias and unpermutes.
"""
import sys
import os

for _p in ("/opt/trn_rl_repo", "/root/.axon_site/_ro/trn_rl_repo"):
    if os.path.isdir(_p) and _p not in sys.path:
        sys.path.insert(0, _p)

import numpy as np
import concourse.bass as bass
import concourse.mybir as mybir
import concourse.tile as tile
from concourse import bacc
from concourse.bass_utils import run_bass_kernel_spmd
from concourse.masks import make_identity

# problem constants (hardcoded per spec)
N = 100000
E = 1600000
E_OUT = 400000
IN_DIM = 16
HID = 64
OUT_DIM = 2
L = 6
BN_EPS = 1e-5

NCORES = 8
NPC_REAL = 12500          # real nodes per core
COLS = 101                # node columns per partition
NPC = 128 * COLS          # 12928 local slots (428 dummies for phantom targets)
TBLR = NCORES * NPC       # 103424 table rows
NWIN = 4                  # gather source windows
WINSZ = TBLR // NWIN      # 25856 (< 32768, int16-safe)
CHUNK = 2048              # max indices per gather/scatter call
MAXCC = CHUNK // 128      # 16 columns per msg tile

f32 = mybir.dt.float32
i16 = mybir.dt.int16

_CACHE = {}


def _pos(v):
    """global node id -> table row."""
    c = v // NPC_REAL
    return c * NPC + (v - c * NPC_REAL)


def _wrap16(idx):
    """[n] int array -> [128, n//16] int16 wrapped+replicated for Ant DMA ops."""
    n = idx.shape[0]
    w = idx.reshape(n // 16, 16).T.astype(np.int16)
    return np.tile(w, (8, 1))


def _ceil(a, b):
    return -(-a // b)


def _plan_chunks(per_core_lists, nwin_bases, tag):
    """per_core_lists: list over cores of (srcpos, scatter_target, passid) arrays
    already sorted by (passid, window, srcpos). Returns a uniform call plan and
    per-core packed idx arrays.

    Each call: same (window, num_idxs) on all cores; cores with fewer real
    entries get phantom entries (gather idx = window base, scatter idx = dummy
    node ids 12500.. which are unique within the call)."""
    # split per core into groups keyed by (passid, window)
    groups = {}  # key -> list over cores of (srcpos_arr, starg_arr)
    for c, (sp, st, pj) in enumerate(per_core_lists):
        w = sp // WINSZ
        key_arr = pj * NWIN + w
        order = np.argsort(key_arr, kind="stable")
        sp, st, key_arr = sp[order], st[order], key_arr[order]
        uk, starts = np.unique(key_arr, return_index=True)
        starts = list(starts) + [len(sp)]
        for i, k in enumerate(uk):
            groups.setdefault(int(k), [[] for _ in range(NCORES)])
            groups[int(k)][c] = (sp[starts[i]:starts[i + 1]], st[starts[i]:starts[i + 1]])
    calls = []   # (window, n_pad, accid)
    packed_g = [[] for _ in range(NCORES)]
    packed_s = [[] for _ in range(NCORES)]
    for key in sorted(groups):
        pj, w = key // NWIN, key % NWIN
        base = w * WINSZ
        percore = groups[key]
        nmax = max(len(x[0]) if x else 0 for x in percore)
        if nmax == 0:
            continue
        ncalls = _ceil(nmax, CHUNK)
        sizes = [min(CHUNK, nmax - i * CHUNK) for i in range(ncalls)]
        for ci in range(ncalls):
            n_pad = _ceil(sizes[ci], 16) * 16
            off = ci * CHUNK
            for c in range(NCORES):
                sp, st = percore[c] if percore[c] else (np.empty(0, np.int64), np.empty(0, np.int64))
                spc = sp[off:off + sizes[ci]]
                stc = st[off:off + sizes[ci]]
                npad = n_pad - len(spc)
                assert npad <= NPC - NPC_REAL, f"{tag}: phantom overflow {npad}"
                g = np.concatenate([spc - base, np.zeros(npad, np.int64)])
                s = np.concatenate([stc, NPC_REAL + np.arange(npad)])
                assert g.min() >= 0 and g.max() < WINSZ
                packed_g[c].append(g)
                packed_s[c].append(s)
            calls.append((w, n_pad, pj % 2))
    return calls, packed_g, packed_s


def _preprocess(edge_index, edge_index_out):
    src = np.asarray(edge_index[0], dtype=np.int64)
    dst = np.asarray(edge_index[1], dtype=np.int64)
    deg = np.bincount(dst, minlength=N).astype(np.float64) + 1.0
    dinv = (1.0 / np.sqrt(deg)).astype(np.float32)

    # per-core edge lists with pass ids (j-th in-edge of each dst, no self-loops)
    per_core = []
    core_of = dst // NPC_REAL
    for c in range(NCORES):
        m = core_of == c
        s_c, d_c = src[m], dst[m] - c * NPC_REAL
        sp = _pos(s_c)
        order = np.lexsort((sp, d_c))
        s_c, d_c, sp = s_c[order], d_c[order], sp[order]
        # pass id = rank within dst group
        first = np.r_[True, d_c[1:] != d_c[:-1]]
        grp_start = np.maximum.accumulate(np.where(first, np.arange(len(d_c)), 0))
        pj = np.arange(len(d_c)) - grp_start
        per_core.append((sp, d_c, pj))
    calls, pg, ps = _plan_chunks(per_core, None, "agg")

    # final-stage: output edges sharded by position
    EPC = E_OUT // NCORES
    fin_lists = []
    fin_edge_maps = []
    for c in range(NCORES):
        es = np.asarray(edge_index_out[0][c * EPC:(c + 1) * EPC], dtype=np.int64)
        ed = np.asarray(edge_index_out[1][c * EPC:(c + 1) * EPC], dtype=np.int64)
        allpos = np.concatenate([_pos(es), _pos(ed)])
        half = np.concatenate([np.zeros(EPC, np.int64), np.ones(EPC, np.int64)])
        eid = np.concatenate([np.arange(EPC), np.arange(EPC)])
        fin_lists.append((allpos, eid, half))
        fin_edge_maps.append((eid, half))
    # reuse the chunk planner: "pass id" := half (0 = src, 1 = dst) so src/dst
    # gathers are separate calls; scatter targets unused here (we keep eid to
    # reconstruct), so feed eid as the scatter array but cap phantom asserts.
    # plan manually (no dst-unique constraint, phantoms just pad)
    fcalls = []
    fpacked = [[] for _ in range(NCORES)]
    fslotmap = [[] for _ in range(NCORES)]  # per call: (edge ids, half flag) per slot
    fg = {}
    for c in range(NCORES):
        allpos, eid, half = fin_lists[c]
        w = allpos // WINSZ
        key = half * NWIN + w
        order = np.lexsort((allpos, key))
        allpos, eid, half, key = allpos[order], eid[order], half[order], key[order]
        uk, starts = np.unique(key, return_index=True)
        starts = list(starts) + [len(allpos)]
        for i, k in enumerate(uk):
            fg.setdefault(int(k), [None] * NCORES)
            fg[int(k)][c] = (allpos[starts[i]:starts[i + 1]], eid[starts[i]:starts[i + 1]],
                             half[starts[i]:starts[i + 1]])
    for key in sorted(fg):
        w = key % NWIN
        base = w * WINSZ
        percore = fg[key]
        nmax = max((len(x[0]) if x is not None else 0) for x in percore)
        if nmax == 0:
            continue
        for ci in range(_ceil(nmax, CHUNK)):
            size = min(CHUNK, nmax - ci * CHUNK)
            n_pad = _ceil(size, 16) * 16
            off = ci * CHUNK
            for c in range(NCORES):
                ap_, eid_, half_ = percore[c] if percore[c] is not None else (
                    np.empty(0, np.int64), np.empty(0, np.int64), np.empty(0, np.int64))
                spc = ap_[off:off + size]
                ei = eid_[off:off + size]
                hf = half_[off:off + size]
                npad = n_pad - len(spc)
                g = np.concatenate([spc - base, np.zeros(npad, np.int64)])
                assert g.min() >= 0 and g.max() < WINSZ
                fpacked[c].append(g)
                fslotmap[c].append((ei, hf))
            fcalls.append((w, n_pad))
    return dinv, calls, pg, ps, fcalls, fpacked, fslotmap


def _build_program(calls, fcalls):
    """Build the SPMD Bass program (identical across cores)."""
    nc = bacc.Bacc("TRN2", target_bir_lowering=False, debug=False,
                   num_devices=NCORES, num_swdge_queues=4)
    GC = sum(n // 16 for _, n, _ in calls)
    FC = sum(n // 16 for _, n in fcalls)
    TOTS = sum(_ceil(n, 128) * 128 for _, n in fcalls)

    xT_in = nc.dram_tensor("xT", [IN_DIM, COLS, 128], f32, kind="ExternalInput")
    dinv_in = nc.dram_tensor("dinv", [128, COLS], f32, kind="ExternalInput")
    wemb_in = nc.dram_tensor("wemb", [IN_DIM, HID], f32, kind="ExternalInput")
    bemb_in = nc.dram_tensor("bemb", [128, HID], f32, kind="ExternalInput")
    convw_in = nc.dram_tensor("convw", [HID, L * HID], f32, kind="ExternalInput")
    bn_in = nc.dram_tensor("bn", [L, 128], f32, kind="ExternalInput")
    fcw_in = nc.dram_tensor("fcw", [HID, 4], f32, kind="ExternalInput")
    gidx_in = nc.dram_tensor("gidx", [128, GC], i16, kind="ExternalInput")
    sidx_in = nc.dram_tensor("sidx", [128, GC], i16, kind="ExternalInput")
    fidx_in = nc.dram_tensor("fidx", [128, FC], i16, kind="ExternalInput")
    yout = nc.dram_tensor("yout", [TOTS, 4], f32, kind="ExternalOutput")

    with tile.TileContext(nc) as tc:
        with (
            tc.tile_pool(name="const", bufs=1) as cp,
            tc.tile_pool(name="big", bufs=1) as bp,
            tc.tile_pool(name="msg", bufs=4) as mp,
            tc.tile_pool(name="work", bufs=2) as wp,
            tc.tile_pool(name="psA", bufs=2, space="PSUM") as psA,
            tc.tile_pool(name="psB", bufs=2, space="PSUM") as psB,
            tc.tile_pool(name="dram", bufs=1, space="DRAM") as dp,
        ):
            # ---- DRAM internals
            bounce = dp.tile([NPC, HID], f32)
            table = dp.tile([TBLR, HID], f32)
            acc0 = dp.tile([NPC, HID], f32, tag="acc0")
            acc1 = dp.tile([NPC, HID], f32, tag="acc1")
            acc = [acc0, acc1]
            arb_in = dp.tile([1, 128], f32)
            arb_out = dp.tile([1, 128], f32)

            # ---- constants / inputs to SBUF
            id128 = cp.tile([128, 128], f32)
            make_identity(nc, id128[:])
            ones_col = cp.tile([128, 1], f32)
            nc.vector.memset(ones_col[:], 1.0)
            ones_row = cp.tile([1, 128], f32)
            nc.vector.memset(ones_row[:], 1.0)
            xT = cp.tile([IN_DIM, COLS, 128], f32)
            nc.sync.dma_start(out=xT[:], in_=xT_in[:])
            dinv_sb = cp.tile([128, COLS], f32)
            nc.sync.dma_start(out=dinv_sb[:], in_=dinv_in[:])
            wemb = cp.tile([IN_DIM, HID], f32)
            nc.sync.dma_start(out=wemb[:], in_=wemb_in[:])
            bemb = cp.tile([128, HID], f32)
            nc.sync.dma_start(out=bemb[:], in_=bemb_in[:])
            convw = cp.tile([HID, L * HID], f32)
            nc.sync.dma_start(out=convw[:], in_=convw_in[:])
            fcw = cp.tile([HID, 4], f32)
            nc.sync.dma_start(out=fcw[:], in_=fcw_in[:])
            gidx = cp.tile([128, GC], i16, tag="gidx")
            nc.sync.dma_start(out=gidx[:], in_=gidx_in[:])
            sidx = cp.tile([128, GC], i16)
            nc.sync.dma_start(out=sidx[:], in_=sidx_in[:])

            _regs = {}

            def reg_of(n):
                if n not in _regs:
                    _regs[n] = nc.gpsimd.to_reg(n)
                return _regs[n]

            xe = bp.tile([128, COLS, HID], f32)     # node-major features
            hwp = bp.tile([128, COLS, HID], f32)    # hw' staging
            agg = bp.tile([128, COLS, HID], f32)

            dinv_b = dinv_sb[:].rearrange("p (c k) -> p c k", k=1).to_broadcast(
                [128, COLS, HID])

            # ---- embed: xe = xT @ wemb + bemb
            for c in range(COLS):
                ps = psA.tile([128, HID], f32, tag="mm")
                nc.tensor.matmul(out=ps[:], lhsT=xT[:, c, :], rhs=wemb[:],
                                 start=True, stop=True)
                nc.vector.tensor_tensor(out=xe[:, c, :], in0=ps[:], in1=bemb[:],
                                        op=mybir.AluOpType.add)

            # ---- layers
            for l in range(L):
                # hw' = (xe @ W_l) * dinv
                for c in range(COLS):
                    pt = psB.tile([64, 128], f32, tag="tr")
                    nc.tensor.transpose(out=pt[:], in_=xe[:, c, :], identity=id128[:])
                    xeT = wp.tile([64, 128], f32, tag="xeT")
                    nc.vector.tensor_copy(out=xeT[:], in_=pt[:])
                    ps = psA.tile([128, HID], f32, tag="mm")
                    nc.tensor.matmul(out=ps[:], lhsT=xeT[:],
                                     rhs=convw[:, l * HID:(l + 1) * HID],
                                     start=True, stop=True)
                    nc.vector.tensor_scalar_mul(out=hwp[:, c, :], in0=ps[:],
                                                scalar1=dinv_sb[:, c:c + 1])
                nc.sync.dma_start(out=bounce[:].rearrange("(p c) k -> p c k", p=128),
                                  in_=hwp[:, :, :])
                nc.gpsimd.collective_compute(
                    "AllGather", mybir.AluOpType.bypass,
                    replica_groups=[list(range(NCORES))],
                    ins=[bounce[:].opt()], outs=[table[:].opt()])

                # zero accumulators (reuse agg tile as the zero source)
                nc.vector.memset(agg[:], 0.0)
                for a in range(2):
                    nc.sync.dma_start(
                        out=acc[a][:].rearrange("(p c) k -> p c k", p=128),
                        in_=agg[:, :, :])

                # gather + scatter chunks
                off = 0
                for k, (w, n_pad, aid) in enumerate(calls):
                    cc = _ceil(n_pad, 128)
                    msg = mp.tile([128, MAXCC, HID], f32, tag="msg")
                    nc.gpsimd.dma_gather(
                        out_ap=msg[:, :cc, :],
                        in_ap=table[w * WINSZ:(w + 1) * WINSZ, :],
                        idxs_ap=gidx[:, off:off + n_pad // 16],
                        num_idxs=n_pad, num_idxs_reg=reg_of(n_pad), elem_size=HID,
                        single_packet=False, queue_num=k % 4)
                    nc.gpsimd.dma_scatter_add(
                        out_ap=acc[aid][:],
                        in_ap=msg[:, :cc, :],
                        idxs_ap=sidx[:, off:off + n_pad // 16],
                        num_idxs=n_pad, num_idxs_reg=reg_of(n_pad), elem_size=HID,
                        single_packet=False, queue_num=k % 4)
                    off += n_pad // 16

                # agg = dinv * (acc0 + acc1 + hw'); hwp is consumed then reused
                nc.sync.dma_start(out=agg[:, :, :],
                                  in_=acc[0][:].rearrange("(p c) k -> p c k", p=128))
                nc.vector.tensor_tensor(out=agg[:], in0=agg[:], in1=hwp[:],
                                        op=mybir.AluOpType.add)
                nc.sync.dma_start(out=hwp[:, :, :],
                                  in_=acc[1][:].rearrange("(p c) k -> p c k", p=128))
                nc.vector.tensor_tensor(out=agg[:], in0=agg[:], in1=hwp[:],
                                        op=mybir.AluOpType.add)
                nc.vector.tensor_tensor(out=agg[:], in0=agg[:], in1=dinv_b,
                                        op=mybir.AluOpType.mult)

                # BN stats: per-channel sums of agg and agg^2
                stats = wp.tile([128, 128], f32, tag="stats")
                nc.vector.tensor_reduce(
                    out=stats[:, 0:HID],
                    in_=agg[:].rearrange("p c k -> p k c"),
                    axis=mybir.AxisListType.X, op=mybir.AluOpType.add)
                nc.vector.tensor_tensor(out=hwp[:], in0=agg[:], in1=agg[:],
                                        op=mybir.AluOpType.mult)
                nc.vector.tensor_reduce(
                    out=stats[:, HID:128],
                    in_=hwp[:].rearrange("p c k -> p k c"),
                    axis=mybir.AxisListType.X, op=mybir.AluOpType.add)
                psS = psB.tile([1, 128], f32, tag="misc")
                nc.tensor.matmul(out=psS[:], lhsT=ones_col[:], rhs=stats[:],
                                 start=True, stop=True)
                sums = wp.tile([1, 128], f32, tag="sums_sb")
                nc.vector.tensor_copy(out=sums[:], in_=psS[:])
                nc.sync.dma_start(out=arb_in[:], in_=sums[:])
                nc.gpsimd.collective_compute(
                    "AllReduce", mybir.AluOpType.add,
                    replica_groups=[list(range(NCORES))],
                    ins=[arb_in[:].opt()], outs=[arb_out[:].opt()])
                gs = wp.tile([1, 128], f32, tag="gs")
                nc.sync.dma_start(out=gs[:], in_=arb_out[:])

                st = wp.tile([1, 128], f32, tag="st")        # scratch: mean|ex2
                nc.vector.tensor_scalar_mul(out=st[:, 0:HID], in0=gs[:, 0:HID],
                                            scalar1=1.0 / N)
                nc.vector.tensor_scalar_mul(out=st[:, HID:128], in0=gs[:, HID:128],
                                            scalar1=1.0 / N)
                var = wp.tile([1, HID], f32, tag="var")
                nc.vector.tensor_tensor(out=var[:], in0=st[:, 0:HID], in1=st[:, 0:HID],
                                        op=mybir.AluOpType.mult)
                nc.vector.tensor_tensor(out=var[:], in0=st[:, HID:128], in1=var[:],
                                        op=mybir.AluOpType.subtract)
                nc.vector.tensor_scalar_add(out=var[:], in0=var[:],
                                            scalar1=float(BN_EPS))
                sd = wp.tile([1, HID], f32, tag="sd")
                nc.scalar.activation(out=sd[:], in_=var[:],
                                     func=mybir.ActivationFunctionType.Sqrt)
                rs = wp.tile([1, HID], f32, tag="rs")
                nc.vector.reciprocal(out=rs[:], in_=sd[:])
                bnl = wp.tile([1, 128], f32, tag="bnl")
                nc.sync.dma_start(out=bnl[:], in_=bn_in[l:l + 1, :])
                stc = wp.tile([1, 128], f32, tag="stc")      # s | t
                nc.vector.tensor_tensor(out=stc[:, 0:HID], in0=bnl[:, 0:HID],
                                        in1=rs[:], op=mybir.AluOpType.mult)
                tmp = wp.tile([1, HID], f32, tag="tmp")
                nc.vector.tensor_tensor(out=tmp[:], in0=st[:, 0:HID],
                                        in1=stc[:, 0:HID], op=mybir.AluOpType.mult)
                nc.vector.tensor_tensor(out=stc[:, HID:128], in0=bnl[:, HID:128],
                                        in1=tmp[:], op=mybir.AluOpType.subtract)
                psBt = psB.tile([128, 128], f32, tag="misc")
                nc.tensor.matmul(out=psBt[:], lhsT=ones_row[:], rhs=stc[:],
                                 start=True, stop=True)
                stb = wp.tile([128, 128], f32, tag="stb")
                nc.vector.tensor_copy(out=stb[:], in_=psBt[:])

                s_b = stb[:, 0:HID].rearrange("p (o k) -> p o k", o=1).to_broadcast(
                    [128, COLS, HID])
                t_b = stb[:, HID:128].rearrange("p (o k) -> p o k", o=1).to_broadcast(
                    [128, COLS, HID])
                nc.vector.tensor_tensor(out=agg[:], in0=agg[:], in1=s_b,
                                        op=mybir.AluOpType.mult)
                nc.vector.tensor_tensor(out=agg[:], in0=agg[:], in1=t_b,
                                        op=mybir.AluOpType.add)
                nc.vector.tensor_scalar_max(out=agg[:], in0=agg[:], scalar1=0.0)
                nc.vector.tensor_tensor(out=xe[:], in0=xe[:], in1=agg[:],
                                        op=mybir.AluOpType.add)

            # ---- final head: y = [xe @ fcw] (4 cols), AllGather, edge gathers
            nc.vector.memset(hwp[:], 0.0)      # reuse hwp as y staging
            for c in range(COLS):
                pt = psB.tile([64, 128], f32, tag="tr")
                nc.tensor.transpose(out=pt[:], in_=xe[:, c, :], identity=id128[:])
                xeT = wp.tile([64, 128], f32, tag="xeT")
                nc.vector.tensor_copy(out=xeT[:], in_=pt[:])
                psY = psA.tile([128, 4], f32, tag="mm")
                nc.tensor.matmul(out=psY[:], lhsT=xeT[:], rhs=fcw[:],
                                 start=True, stop=True)
                nc.vector.tensor_copy(out=hwp[:, c, 0:4], in_=psY[:])
            nc.sync.dma_start(out=bounce[:].rearrange("(p c) k -> p c k", p=128),
                              in_=hwp[:, :, :])
            nc.gpsimd.collective_compute(
                "AllGather", mybir.AluOpType.bypass,
                replica_groups=[list(range(NCORES))],
                ins=[bounce[:].opt()], outs=[table[:].opt()])

            fidx = cp.tile([128, GC], i16, tag="gidx")
            nc.sync.dma_start(out=fidx[:, :FC], in_=fidx_in[:])
            off = 0
            soff = 0
            for k, (w, n_pad) in enumerate(fcalls):
                cc = _ceil(n_pad, 128)
                msg = mp.tile([128, MAXCC, HID], f32, tag="msg")
                nc.gpsimd.dma_gather(
                    out_ap=msg[:, :cc, :],
                    in_ap=table[w * WINSZ:(w + 1) * WINSZ, :],
                    idxs_ap=fidx[:, off:off + n_pad // 16],
                    num_idxs=n_pad, num_idxs_reg=reg_of(n_pad), elem_size=HID,
                    single_packet=False, queue_num=k % 4)
                yo = mp.tile([128, MAXCC, 4], f32, tag="yo")
                nc.vector.tensor_copy(out=yo[:, :cc, :], in_=msg[:, :cc, 0:4])
                nc.sync.dma_start(
                    out=yout[soff:soff + cc * 128, :].rearrange(
                        "(p c) k -> p c k", p=128),
                    in_=yo[:, :cc, :])
                off += n_pad // 16
                soff += cc * 128
    nc.compile()
    return nc, TOTS


def _prepare(inputs):
    edge_index = np.asarray(inputs["edge_index"])
    edge_index_out = np.asarray(inputs["edge_index_out"])
    key = hash((edge_index[0, :50].tobytes(), edge_index_out[0, :50].tobytes()))
    if key in _CACHE:
        return _CACHE[key]
    dinv, calls, pg, ps, fcalls, fpacked, fslotmap = _preprocess(
        edge_index, edge_index_out)
    nc, TOTS = _build_program(calls, fcalls)
    # pack per-core idx tensors
    gidx_np, sidx_np, fidx_np = [], [], []
    for c in range(NCORES):
        gidx_np.append(np.concatenate([_wrap16(a) for a in pg[c]], axis=1))
        sidx_np.append(np.concatenate([_wrap16(a) for a in ps[c]], axis=1))
        fidx_np.append(np.concatenate([_wrap16(a) for a in fpacked[c]], axis=1))
    _CACHE[key] = (dinv, calls, fcalls, fslotmap, nc, TOTS,
                   gidx_np, sidx_np, fidx_np)
    return _CACHE[key]


def kernel(x, edge_index, edge_index_out, W_emb, b_emb, conv_W, conv_b,
           bn_gamma, bn_beta, fc_W, fc_b):
    (dinv, calls, fcalls, fslotmap, nc, TOTS,
     gidx_np, sidx_np, fidx_np) = _prepare(dict(edge_index=edge_index,
                                                edge_index_out=edge_index_out))
    x = np.asarray(x, np.float32)
    W_emb = np.asarray(W_emb, np.float32)
    b_emb = np.asarray(b_emb, np.float32)
    conv_W = np.asarray(conv_W, np.float32)
    bn_gamma = np.asarray(bn_gamma, np.float32)
    bn_beta = np.asarray(bn_beta, np.float32)
    fc_W = np.asarray(fc_W, np.float32)
    fc_b = np.asarray(fc_b, np.float32)

    fcw_cat = np.concatenate([fc_W[:HID], fc_W[HID:]], axis=1)  # [64, 4]
    bn_cat = np.concatenate([bn_gamma, bn_beta], axis=1)        # [L, 128]
    in_maps = []
    for c in range(NCORES):
        xs = np.zeros((NPC, IN_DIM), np.float32)
        xs[:NPC_REAL] = x[c * NPC_REAL:(c + 1) * NPC_REAL]
        dv = np.zeros(NPC, np.float32)
        dv[:NPC_REAL] = dinv[c * NPC_REAL:(c + 1) * NPC_REAL]
        # local id n = p*COLS + cc  ->  xT[:, cc, p] = x[n]
        xT = xs.reshape(128, COLS, IN_DIM).transpose(2, 1, 0).copy()
        dinv_pc = dv.reshape(128, COLS)
        in_maps.append(dict(
            xT=np.ascontiguousarray(xT),
            dinv=np.ascontiguousarray(dinv_pc),
            wemb=W_emb, bemb=np.tile(b_emb[None, :], (128, 1)),
            convw=np.ascontiguousarray(np.transpose(conv_W, (1, 0, 2)).reshape(HID, L * HID)), bn=bn_cat, fcw=fcw_cat,
            gidx=gidx_np[c], sidx=sidx_np[c], fidx=fidx_np[c],
        ))
    res = run_bass_kernel_spmd(nc, in_maps, core_ids=list(range(NCORES)))

    # host: reassemble output
    EPC = E_OUT // NCORES
    out = np.zeros((E_OUT, OUT_DIM), np.float32)
    for c in range(NCORES):
        y = res.results[c]["yout"]            # [TOTS, 4]
        soff = 0
        for k, (w, n_pad) in enumerate(fcalls):
            cc = _ceil(n_pad, 128)
            eid, half = fslotmap[c][k]
            nreal = len(eid)
            # slot i (gather order) -> yout row soff + (i%128)*cc + i//128
            i = np.arange(nreal)
            rows = soff + (i % 128) * cc + i // 128
            vals = y[rows]                    # [nreal, 4]
            sel_src = half == 0
            out[c * EPC + eid[sel_src], :] += vals[sel_src][:, 0:2]
            out[c * EPC + eid[~sel_src], :] += vals[~sel_src][:, 2:4]
            soff += cc * 128
    out += fc_b[None, :]
    return out


# revision 14
# speedup vs baseline: 139.0144x; 139.0144x over previous
"""8-core TRN2 Bass kernel for the 6-layer GCN edge classifier.

Architecture (per core, nodes dst-sharded 8 ways):
- hw' = (x_embed @ W_l) * dinv  computed locally, AllGather -> replicated table
- message aggregation: dma_gather (src-window int16-rebased chunks, 4 SWDGE
  queues) + dma_scatter_add into HBM accumulators; chunks are "pass"-structured
  so destination indices are unique within every scatter call (passes = j-th
  in-edge of each node), alternating 2 accumulators for pipelining.
- BN stats via on-chip reduction + tiny AllReduce; apply+relu+residual on DVE.
- final edge head: y = [xe@fcW_top | xe@fcW_bot] per node -> AllGather ->
  per-edge gathers; host adds the two halves + bias and unpermutes.
"""
import sys
import os

for _p in ("/opt/trn_rl_repo", "/root/.axon_site/_ro/trn_rl_repo"):
    if os.path.isdir(_p) and _p not in sys.path:
        sys.path.insert(0, _p)

import numpy as np
import concourse.bass as bass
import concourse.mybir as mybir
import concourse.tile as tile
from concourse import bacc
from concourse.bass_utils import run_bass_kernel_spmd
from concourse.masks import make_identity

# problem constants (hardcoded per spec)
N = 100000
E = 1600000
E_OUT = 400000
IN_DIM = 16
HID = 64
OUT_DIM = 2
L = 6
BN_EPS = 1e-5

NCORES = 8
NPC_REAL = 12500          # real nodes per core
COLS = 101                # node columns per partition
NPC = 128 * COLS          # 12928 local slots (428 dummies for phantom targets)
TBLR = NCORES * NPC       # 103424 table rows
NWIN = 4                  # gather source windows
WINSZ = TBLR // NWIN      # 25856 (< 32768, int16-safe)
CHUNK = 2048              # max indices per gather/scatter call
MAXCC = CHUNK // 128      # 16 columns per msg tile

f32 = mybir.dt.float32
i16 = mybir.dt.int16

_CACHE = {}


def _pos(v):
    """global node id -> table row."""
    c = v // NPC_REAL
    return c * NPC + (v - c * NPC_REAL)


def _wrap16(idx):
    """[n] int array -> [128, n//16] int16 wrapped+replicated for Ant DMA ops."""
    n = idx.shape[0]
    w = idx.reshape(n // 16, 16).T.astype(np.int16)
    return np.tile(w, (8, 1))


def _ceil(a, b):
    return -(-a // b)


def _plan_chunks(per_core_lists, nwin_bases, tag):
    """per_core_lists: list over cores of (srcpos, scatter_target, passid) arrays
    already sorted by (passid, window, srcpos). Returns a uniform call plan and
    per-core packed idx arrays.

    Each call: same (window, num_idxs) on all cores; cores with fewer real
    entries get phantom entries (gather idx = window base, scatter idx = dummy
    node ids 12500.. which are unique within the call)."""
    # split per core into groups keyed by (passid, window)
    groups = {}  # key -> list over cores of (srcpos_arr, starg_arr)
    for c, (sp, st, pj) in enumerate(per_core_lists):
        w = sp // WINSZ
        key_arr = pj * NWIN + w
        order = np.argsort(key_arr, kind="stable")
        sp, st, key_arr = sp[order], st[order], key_arr[order]
        uk, starts = np.unique(key_arr, return_index=True)
        starts = list(starts) + [len(sp)]
        for i, k in enumerate(uk):
            groups.setdefault(int(k), [[] for _ in range(NCORES)])
            groups[int(k)][c] = (sp[starts[i]:starts[i + 1]], st[starts[i]:starts[i + 1]])
    calls = []   # (window, n_pad, accid)
    packed_g = [[] for _ in range(NCORES)]
    packed_s = [[] for _ in range(NCORES)]
    for key in sorted(groups):
        pj, w = key // NWIN, key % NWIN
        base = w * WINSZ
        percore = groups[key]
        nmax = max(len(x[0]) if x else 0 for x in percore)
        if nmax == 0:
            continue
        ncalls = _ceil(nmax, CHUNK)
        sizes = [min(CHUNK, nmax - i * CHUNK) for i in range(ncalls)]
        for ci in range(ncalls):
            n_pad = _ceil(sizes[ci], 16) * 16
            off = ci * CHUNK
            for c in range(NCORES):
                sp, st = percore[c] if percore[c] else (np.empty(0, np.int64), np.empty(0, np.int64))
                spc = sp[off:off + sizes[ci]]
                stc = st[off:off + sizes[ci]]
                npad = n_pad - len(spc)
                assert npad <= NPC - NPC_REAL, f"{tag}: phantom overflow {npad}"
                g = np.concatenate([spc - base, np.zeros(npad, np.int64)])
                s = np.concatenate([stc, NPC_REAL + np.arange(npad)])
                assert g.min() >= 0 and g.max() < WINSZ
                packed_g[c].append(g)
                packed_s[c].append(s)
            calls.append((w, n_pad, pj % 4))
    return calls, packed_g, packed_s


def _preprocess(edge_index, edge_index_out):
    src = np.asarray(edge_index[0], dtype=np.int64)
    dst = np.asarray(edge_index[1], dtype=np.int64)
    deg = np.bincount(dst, minlength=N).astype(np.float64) + 1.0
    dinv = (1.0 / np.sqrt(deg)).astype(np.float32)

    # per-core edge lists with pass ids (j-th in-edge of each dst, no self-loops)
    per_core = []
    core_of = dst // NPC_REAL
    for c in range(NCORES):
        m = core_of == c
        s_c, d_c = src[m], dst[m] - c * NPC_REAL
        sp = _pos(s_c)
        order = np.lexsort((sp, d_c))
        s_c, d_c, sp = s_c[order], d_c[order], sp[order]
        # pass id = rank within dst group
        first = np.r_[True, d_c[1:] != d_c[:-1]]
        grp_start = np.maximum.accumulate(np.where(first, np.arange(len(d_c)), 0))
        pj = np.arange(len(d_c)) - grp_start
        per_core.append((sp, d_c, pj))
    calls, pg, ps = _plan_chunks(per_core, None, "agg")

    # final-stage: output edges sharded by position
    EPC = E_OUT // NCORES
    fin_lists = []
    fin_edge_maps = []
    for c in range(NCORES):
        es = np.asarray(edge_index_out[0][c * EPC:(c + 1) * EPC], dtype=np.int64)
        ed = np.asarray(edge_index_out[1][c * EPC:(c + 1) * EPC], dtype=np.int64)
        allpos = np.concatenate([_pos(es), _pos(ed)])
        half = np.concatenate([np.zeros(EPC, np.int64), np.ones(EPC, np.int64)])
        eid = np.concatenate([np.arange(EPC), np.arange(EPC)])
        fin_lists.append((allpos, eid, half))
        fin_edge_maps.append((eid, half))
    # reuse the chunk planner: "pass id" := half (0 = src, 1 = dst) so src/dst
    # gathers are separate calls; scatter targets unused here (we keep eid to
    # reconstruct), so feed eid as the scatter array but cap phantom asserts.
    # plan manually (no dst-unique constraint, phantoms just pad)
    fcalls = []
    fpacked = [[] for _ in range(NCORES)]
    fslotmap = [[] for _ in range(NCORES)]  # per call: (edge ids, half flag) per slot
    fg = {}
    for c in range(NCORES):
        allpos, eid, half = fin_lists[c]
        w = allpos // WINSZ
        key = half * NWIN + w
        order = np.lexsort((allpos, key))
        allpos, eid, half, key = allpos[order], eid[order], half[order], key[order]
        uk, starts = np.unique(key, return_index=True)
        starts = list(starts) + [len(allpos)]
        for i, k in enumerate(uk):
            fg.setdefault(int(k), [None] * NCORES)
            fg[int(k)][c] = (allpos[starts[i]:starts[i + 1]], eid[starts[i]:starts[i + 1]],
                             half[starts[i]:starts[i + 1]])
    for key in sorted(fg):
        w = key % NWIN
        base = w * WINSZ
        percore = fg[key]
        nmax = max((len(x[0]) if x is not None else 0) for x in percore)
        if nmax == 0:
            continue
        for ci in range(_ceil(nmax, CHUNK)):
            size = min(CHUNK, nmax - ci * CHUNK)
            n_pad = _ceil(size, 16) * 16
            off = ci * CHUNK
            for c in range(NCORES):
                ap_, eid_, half_ = percore[c] if percore[c] is not None else (
                    np.empty(0, np.int64), np.empty(0, np.int64), np.empty(0, np.int64))
                spc = ap_[off:off + size]
                ei = eid_[off:off + size]
                hf = half_[off:off + size]
                npad = n_pad - len(spc)
                g = np.concatenate([spc - base, np.zeros(npad, np.int64)])
                assert g.min() >= 0 and g.max() < WINSZ
                fpacked[c].append(g)
                fslotmap[c].append((ei, hf))
            fcalls.append((w, n_pad))
    return dinv, calls, pg, ps, fcalls, fpacked, fslotmap


def _build_program(calls, fcalls, repeat=1, no_scatter=False, body=True):
    """Build the SPMD Bass program (identical across cores)."""
    nc = bacc.Bacc("TRN2", target_bir_lowering=False, debug=False,
                   num_devices=NCORES, num_swdge_queues=4)
    GC = sum(n // 16 for _, n, _ in calls)
    FC = sum(n // 16 for _, n in fcalls)
    TOTS = sum(_ceil(n, 128) * 128 for _, n in fcalls)

    xT_in = nc.dram_tensor("xT", [IN_DIM, COLS, 128], f32, kind="ExternalInput")
    dinv_in = nc.dram_tensor("dinv", [128, COLS], f32, kind="ExternalInput")
    wemb_in = nc.dram_tensor("wemb", [IN_DIM, HID], f32, kind="ExternalInput")
    bemb_in = nc.dram_tensor("bemb", [128, HID], f32, kind="ExternalInput")
    convw_in = nc.dram_tensor("convw", [HID, L * HID], f32, kind="ExternalInput")
    bn_in = nc.dram_tensor("bn", [L, 128], f32, kind="ExternalInput")
    fcw_in = nc.dram_tensor("fcw", [HID, 4], f32, kind="ExternalInput")
    gidx_in = nc.dram_tensor("gidx", [128, GC], i16, kind="ExternalInput")
    sidx_in = nc.dram_tensor("sidx", [128, GC], i16, kind="ExternalInput")
    fidx_in = nc.dram_tensor("fidx", [128, FC], i16, kind="ExternalInput")
    yout = nc.dram_tensor("yout", [TOTS, 4], f32, kind="ExternalOutput")

    with tile.TileContext(nc) as tc:
        with (
            tc.tile_pool(name="const", bufs=1) as cp,
            tc.tile_pool(name="big", bufs=1) as bp,
            tc.tile_pool(name="msg", bufs=4) as mp,
            tc.tile_pool(name="work", bufs=2) as wp,
            tc.tile_pool(name="psA", bufs=2, space="PSUM") as psA,
            tc.tile_pool(name="psB", bufs=2, space="PSUM") as psB,
            tc.tile_pool(name="dram", bufs=1, space="DRAM") as dp,
        ):
            # ---- DRAM internals
            bounce = dp.tile([NPC, HID], f32)
            table = dp.tile([TBLR, HID], f32)
            acc0 = dp.tile([NPC, HID], f32, tag="acc0")
            acc1 = dp.tile([NPC, HID], f32, tag="acc1")
            acc2 = dp.tile([NPC, HID], f32, tag="acc2")
            acc3 = dp.tile([NPC, HID], f32, tag="acc3")
            acc = [acc0, acc1, acc2, acc3]
            arb_in = dp.tile([1, 128], f32)
            arb_out = dp.tile([1, 128], f32)

            # ---- constants / inputs to SBUF
            id128 = cp.tile([128, 128], f32)
            make_identity(nc, id128[:])
            ones_col = cp.tile([128, 1], f32)
            nc.vector.memset(ones_col[:], 1.0)
            ones_row = cp.tile([1, 128], f32)
            nc.vector.memset(ones_row[:], 1.0)
            xT = cp.tile([IN_DIM, COLS, 128], f32)
            nc.sync.dma_start(out=xT[:], in_=xT_in[:])
            dinv_sb = cp.tile([128, COLS], f32)
            nc.sync.dma_start(out=dinv_sb[:], in_=dinv_in[:])
            wemb = cp.tile([IN_DIM, HID], f32)
            nc.sync.dma_start(out=wemb[:], in_=wemb_in[:])
            bemb = cp.tile([128, HID], f32)
            nc.sync.dma_start(out=bemb[:], in_=bemb_in[:])
            convw = cp.tile([HID, L * HID], f32)
            nc.sync.dma_start(out=convw[:], in_=convw_in[:])
            fcw = cp.tile([HID, 4], f32)
            nc.sync.dma_start(out=fcw[:], in_=fcw_in[:])
            gidx = cp.tile([128, GC], i16, tag="gidx")
            nc.sync.dma_start(out=gidx[:], in_=gidx_in[:])
            sidx = cp.tile([128, GC], i16)
            nc.sync.dma_start(out=sidx[:], in_=sidx_in[:])

            _regs = {}

            def reg_of(n):
                if n not in _regs:
                    _regs[n] = nc.gpsimd.to_reg(n)
                return _regs[n]

            xe = bp.tile([128, COLS, HID], f32)     # node-major features
            hwp = bp.tile([128, COLS, HID], f32)    # hw' staging
            agg = bp.tile([128, COLS, HID], f32)

            dinv_b = dinv_sb[:].rearrange("p (c k) -> p c k", k=1).to_broadcast(
                [128, COLS, HID])

            # ---- embed: xe = xT @ wemb + bemb
            for c in range(COLS if body else 0):
                ps = psA.tile([128, HID], f32, tag="mm")
                nc.tensor.matmul(out=ps[:], lhsT=xT[:, c, :], rhs=wemb[:],
                                 start=True, stop=True)
                nc.vector.tensor_tensor(out=xe[:, c, :], in0=ps[:], in1=bemb[:],
                                        op=mybir.AluOpType.add)

            # ---- layers
            for l in ([li for _ in range(repeat) for li in range(L)] if body else []):
                # hw' = (xe @ W_l) * dinv
                for c in range(COLS):
                    pt = psB.tile([64, 128], f32, tag="tr")
                    nc.tensor.transpose(out=pt[:], in_=xe[:, c, :], identity=id128[:])
                    xeT = wp.tile([64, 128], f32, tag="xeT")
                    nc.vector.tensor_copy(out=xeT[:], in_=pt[:])
                    ps = psA.tile([128, HID], f32, tag="mm")
                    nc.tensor.matmul(out=ps[:], lhsT=xeT[:],
                                     rhs=convw[:, l * HID:(l + 1) * HID],
                                     start=True, stop=True)
                    nc.vector.tensor_scalar_mul(out=hwp[:, c, :], in0=ps[:],
                                                scalar1=dinv_sb[:, c:c + 1])
                nc.sync.dma_start(out=bounce[:].rearrange("(p c) k -> p c k", p=128),
                                  in_=hwp[:, :, :])
                nc.gpsimd.collective_compute(
                    "AllGather", mybir.AluOpType.bypass,
                    replica_groups=[list(range(NCORES))],
                    ins=[bounce[:].opt()], outs=[table[:].opt()])

                # zero accumulators (reuse agg tile as the zero source)
                nc.vector.memset(agg[:], 0.0)
                for a in range(4):
                    nc.sync.dma_start(
                        out=acc[a][:].rearrange("(p c) k -> p c k", p=128),
                        in_=agg[:, :, :])

                # gather + scatter chunks
                off = 0
                for k, (w, n_pad, aid) in enumerate(calls):
                    cc = _ceil(n_pad, 128)
                    msg = mp.tile([128, MAXCC, HID], f32, tag="msg")
                    nc.gpsimd.dma_gather(
                        out_ap=msg[:, :cc, :],
                        in_ap=table[w * WINSZ:(w + 1) * WINSZ, :],
                        idxs_ap=gidx[:, off:off + n_pad // 16],
                        num_idxs=n_pad, num_idxs_reg=reg_of(n_pad), elem_size=HID,
                        single_packet=False, queue_num=k % 2)
                    if no_scatter:
                        off += n_pad // 16
                        continue
                    nc.gpsimd.dma_scatter_add(
                        out_ap=acc[aid][:],
                        in_ap=msg[:, :cc, :],
                        idxs_ap=sidx[:, off:off + n_pad // 16],
                        num_idxs=n_pad, num_idxs_reg=reg_of(n_pad), elem_size=HID,
                        single_packet=False, queue_num=2 + k % 2)
                    off += n_pad // 16

                # agg = dinv * (acc0 + acc1 + hw'); hwp is consumed then reused
                nc.sync.dma_start(out=agg[:, :, :],
                                  in_=acc[0][:].rearrange("(p c) k -> p c k", p=128))
                nc.vector.tensor_tensor(out=agg[:], in0=agg[:], in1=hwp[:],
                                        op=mybir.AluOpType.add)
                for a in range(1, 4):
                    nc.sync.dma_start(out=hwp[:, :, :],
                                      in_=acc[a][:].rearrange("(p c) k -> p c k", p=128))
                    nc.vector.tensor_tensor(out=agg[:], in0=agg[:], in1=hwp[:],
                                            op=mybir.AluOpType.add)
                nc.vector.tensor_tensor(out=agg[:], in0=agg[:], in1=dinv_b,
                                        op=mybir.AluOpType.mult)

                # BN stats: per-channel sums of agg and agg^2
                stats = wp.tile([128, 128], f32, tag="stats")
                nc.vector.tensor_reduce(
                    out=stats[:, 0:HID],
                    in_=agg[:].rearrange("p c k -> p k c"),
                    axis=mybir.AxisListType.X, op=mybir.AluOpType.add)
                nc.vector.tensor_tensor(out=hwp[:], in0=agg[:], in1=agg[:],
                                        op=mybir.AluOpType.mult)
                nc.vector.tensor_reduce(
                    out=stats[:, HID:128],
                    in_=hwp[:].rearrange("p c k -> p k c"),
                    axis=mybir.AxisListType.X, op=mybir.AluOpType.add)
                psS = psB.tile([1, 128], f32, tag="misc")
                nc.tensor.matmul(out=psS[:], lhsT=ones_col[:], rhs=stats[:],
                                 start=True, stop=True)
                sums = wp.tile([1, 128], f32, tag="sums_sb")
                nc.vector.tensor_copy(out=sums[:], in_=psS[:])
                nc.sync.dma_start(out=arb_in[:], in_=sums[:])
                nc.gpsimd.collective_compute(
                    "AllReduce", mybir.AluOpType.add,
                    replica_groups=[list(range(NCORES))],
                    ins=[arb_in[:].opt()], outs=[arb_out[:].opt()])
                gs = wp.tile([1, 128], f32, tag="gs")
                nc.sync.dma_start(out=gs[:], in_=arb_out[:])

                st = wp.tile([1, 128], f32, tag="st")        # scratch: mean|ex2
                nc.vector.tensor_scalar_mul(out=st[:, 0:HID], in0=gs[:, 0:HID],
                                            scalar1=1.0 / N)
                nc.vector.tensor_scalar_mul(out=st[:, HID:128], in0=gs[:, HID:128],
                                            scalar1=1.0 / N)
                var = wp.tile([1, HID], f32, tag="var")
                nc.vector.tensor_tensor(out=var[:], in0=st[:, 0:HID], in1=st[:, 0:HID],
                                        op=mybir.AluOpType.mult)
                nc.vector.tensor_tensor(out=var[:], in0=st[:, HID:128], in1=var[:],
                                        op=mybir.AluOpType.subtract)
                nc.vector.tensor_scalar_add(out=var[:], in0=var[:],
                                            scalar1=float(BN_EPS))
                sd = wp.tile([1, HID], f32, tag="sd")
                nc.scalar.activation(out=sd[:], in_=var[:],
                                     func=mybir.ActivationFunctionType.Sqrt)
                rs = wp.tile([1, HID], f32, tag="rs")
                nc.vector.reciprocal(out=rs[:], in_=sd[:])
                bnl = wp.tile([1, 128], f32, tag="bnl")
                nc.sync.dma_start(out=bnl[:], in_=bn_in[l:l + 1, :])
                stc = wp.tile([1, 128], f32, tag="stc")      # s | t
                nc.vector.tensor_tensor(out=stc[:, 0:HID], in0=bnl[:, 0:HID],
                                        in1=rs[:], op=mybir.AluOpType.mult)
                tmp = wp.tile([1, HID], f32, tag="tmp")
                nc.vector.tensor_tensor(out=tmp[:], in0=st[:, 0:HID],
                                        in1=stc[:, 0:HID], op=mybir.AluOpType.mult)
                nc.vector.tensor_tensor(out=stc[:, HID:128], in0=bnl[:, HID:128],
                                        in1=tmp[:], op=mybir.AluOpType.subtract)
                psBt = psB.tile([128, 128], f32, tag="misc")
                nc.tensor.matmul(out=psBt[:], lhsT=ones_row[:], rhs=stc[:],
                                 start=True, stop=True)
                stb = wp.tile([128, 128], f32, tag="stb")
                nc.vector.tensor_copy(out=stb[:], in_=psBt[:])

                s_b = stb[:, 0:HID].rearrange("p (o k) -> p o k", o=1).to_broadcast(
                    [128, COLS, HID])
                t_b = stb[:, HID:128].rearrange("p (o k) -> p o k", o=1).to_broadcast(
                    [128, COLS, HID])
                nc.vector.tensor_tensor(out=agg[:], in0=agg[:], in1=s_b,
                                        op=mybir.AluOpType.mult)
                nc.vector.tensor_tensor(out=agg[:], in0=agg[:], in1=t_b,
                                        op=mybir.AluOpType.add)
                nc.vector.tensor_scalar_max(out=agg[:], in0=agg[:], scalar1=0.0)
                nc.vector.tensor_tensor(out=xe[:], in0=xe[:], in1=agg[:],
                                        op=mybir.AluOpType.add)

            # ---- final head: y = [xe @ fcw] (4 cols), AllGather, edge gathers
            nc.vector.memset(hwp[:], 0.0)      # reuse hwp as y staging
            for c in range(COLS if body else 0):
                pt = psB.tile([64, 128], f32, tag="tr")
                nc.tensor.transpose(out=pt[:], in_=xe[:, c, :], identity=id128[:])
                xeT = wp.tile([64, 128], f32, tag="xeT")
                nc.vector.tensor_copy(out=xeT[:], in_=pt[:])
                psY = psA.tile([128, 4], f32, tag="mm")
                nc.tensor.matmul(out=psY[:], lhsT=xeT[:], rhs=fcw[:],
                                 start=True, stop=True)
                nc.vector.tensor_copy(out=hwp[:, c, 0:4], in_=psY[:])
            nc.sync.dma_start(out=bounce[:].rearrange("(p c) k -> p c k", p=128),
                              in_=hwp[:, :, :])
            nc.gpsimd.collective_compute(
                "AllGather", mybir.AluOpType.bypass,
                replica_groups=[list(range(NCORES))],
                ins=[bounce[:].opt()], outs=[table[:].opt()])

            fidx = cp.tile([128, GC], i16, tag="gidx")
            nc.sync.dma_start(out=fidx[:, :FC], in_=fidx_in[:])
            off = 0
            soff = 0
            for k, (w, n_pad) in enumerate(fcalls if body else []):
                cc = _ceil(n_pad, 128)
                msg = mp.tile([128, MAXCC, HID], f32, tag="msg")
                nc.gpsimd.dma_gather(
                    out_ap=msg[:, :cc, :],
                    in_ap=table[w * WINSZ:(w + 1) * WINSZ, :],
                    idxs_ap=fidx[:, off:off + n_pad // 16],
                    num_idxs=n_pad, num_idxs_reg=reg_of(n_pad), elem_size=HID,
                    single_packet=False, queue_num=k % 4)
                yo = mp.tile([128, MAXCC, 4], f32, tag="yo")
                nc.vector.tensor_copy(out=yo[:, :cc, :], in_=msg[:, :cc, 0:4])
                nc.sync.dma_start(
                    out=yout[soff:soff + cc * 128, :].rearrange(
                        "(p c) k -> p c k", p=128),
                    in_=yo[:, :cc, :])
                off += n_pad // 16
                soff += cc * 128
            if not body:
                yo0 = mp.tile([128, MAXCC, 4], f32, tag="yo")
                nc.vector.memset(yo0[:], 0.0)
                nc.sync.dma_start(
                    out=yout[0:MAXCC * 128, :].rearrange("(p c) k -> p c k", p=128),
                    in_=yo0[:, :, :])
    nc.compile()
    return nc, TOTS


def _prepare(inputs):
    edge_index = np.asarray(inputs["edge_index"])
    edge_index_out = np.asarray(inputs["edge_index_out"])
    key = hash((edge_index[0, :50].tobytes(), edge_index_out[0, :50].tobytes()))
    if key in _CACHE:
        return _CACHE[key]
    dinv, calls, pg, ps, fcalls, fpacked, fslotmap = _preprocess(
        edge_index, edge_index_out)
    nc, TOTS = _build_program(calls, fcalls)
    # pack per-core idx tensors
    gidx_np, sidx_np, fidx_np = [], [], []
    for c in range(NCORES):
        gidx_np.append(np.concatenate([_wrap16(a) for a in pg[c]], axis=1))
        sidx_np.append(np.concatenate([_wrap16(a) for a in ps[c]], axis=1))
        fidx_np.append(np.concatenate([_wrap16(a) for a in fpacked[c]], axis=1))
    _CACHE[key] = (dinv, calls, fcalls, fslotmap, nc, TOTS,
                   gidx_np, sidx_np, fidx_np)
    return _CACHE[key]


def kernel(x, edge_index, edge_index_out, W_emb, b_emb, conv_W, conv_b,
           bn_gamma, bn_beta, fc_W, fc_b):
    (dinv, calls, fcalls, fslotmap, nc, TOTS,
     gidx_np, sidx_np, fidx_np) = _prepare(dict(edge_index=edge_index,
                                                edge_index_out=edge_index_out))
    x = np.asarray(x, np.float32)
    W_emb = np.asarray(W_emb, np.float32)
    b_emb = np.asarray(b_emb, np.float32)
    conv_W = np.asarray(conv_W, np.float32)
    bn_gamma = np.asarray(bn_gamma, np.float32)
    bn_beta = np.asarray(bn_beta, np.float32)
    fc_W = np.asarray(fc_W, np.float32)
    fc_b = np.asarray(fc_b, np.float32)

    fcw_cat = np.concatenate([fc_W[:HID], fc_W[HID:]], axis=1)  # [64, 4]
    bn_cat = np.concatenate([bn_gamma, bn_beta], axis=1)        # [L, 128]
    in_maps = []
    for c in range(NCORES):
        xs = np.zeros((NPC, IN_DIM), np.float32)
        xs[:NPC_REAL] = x[c * NPC_REAL:(c + 1) * NPC_REAL]
        dv = np.zeros(NPC, np.float32)
        dv[:NPC_REAL] = dinv[c * NPC_REAL:(c + 1) * NPC_REAL]
        # local id n = p*COLS + cc  ->  xT[:, cc, p] = x[n]
        xT = xs.reshape(128, COLS, IN_DIM).transpose(2, 1, 0).copy()
        dinv_pc = dv.reshape(128, COLS)
        in_maps.append(dict(
            xT=np.ascontiguousarray(xT),
            dinv=np.ascontiguousarray(dinv_pc),
            wemb=W_emb, bemb=np.tile(b_emb[None, :], (128, 1)),
            convw=np.ascontiguousarray(np.transpose(conv_W, (1, 0, 2)).reshape(HID, L * HID)), bn=bn_cat, fcw=fcw_cat,
            gidx=gidx_np[c], sidx=sidx_np[c], fidx=fidx_np[c],
        ))
    res = run_bass_kernel_spmd(nc, in_maps, core_ids=list(range(NCORES)))

    # host: reassemble output
    EPC = E_OUT // NCORES
    out = np.zeros((E_OUT, OUT_DIM), np.float32)
    for c in range(NCORES):
        y = res.results[c]["yout"]            # [TOTS, 4]
        soff = 0
        for k, (w, n_pad) in enumerate(fcalls):
            cc = _ceil(n_pad, 128)
            eid, half = fslotmap[c][k]
            nreal = len(eid)
            # slot i (gather order) -> yout row soff + (i%128)*cc + i//128
            i = np.arange(nreal)
            rows = soff + (i % 128) * cc + i // 128
            vals = y[rows]                    # [nreal, 4]
            sel_src = half == 0
            out[c * EPC + eid[sel_src], :] += vals[sel_src][:, 0:2]
            out[c * EPC + eid[~sel_src], :] += vals[~sel_src][:, 2:4]
            soff += cc * 128
    out += fc_b[None, :]
    return out
